# revision 2
# baseline (speedup 1.0000x reference)
"""DualGNNModel Trainium2 kernel (8 NeuronCores, Bass/Tile).

Self-contained: accepts FULL inputs (as reference.setup_inputs()), returns the
FULL [256, 1] float32 output.

Sharding: cores 0-3 run the solute GCN encoder, cores 4-7 the solvent encoder
(graph-level model parallelism over the two independent encoders). Within each
4-core group, edges are partitioned by destination node into 4 contiguous
12800-node ranges; GCN/MLP weights are replicated. Per layer each core:
  dense   b = h @ W                  (replicated over the group)
  gather  g = b[src] rows            (dma_gather, two int16 half-table passes)
  scale   g *= nsrc[src]*ndst[dst]   (symmetric GCN norm folded per edge)
  scatter psum[feat, dstwin] += g_chunk^T @ onehot(dst)    (PE matmuls)
  bias    psum += outer(bias, ones);  hT_own = relu(psum)
h shards are AllGather'd between layers; after layer 3 each core pools its own
shard via a membership-one-hot matmul, partial pools are AllReduce'd within the
group, the two encoders' pooled embeddings are exchanged pairwise, and the
(tiny) MLP head runs replicated on every core.
"""
import numpy as np
import concourse.bass as bass
import concourse.bacc as bacc
import concourse.mybir as mybir
import concourse.tile as tile
from concourse.library_config import mlp as mlp_lib
from concourse.masks import make_identity
from concourse.bass_utils import run_bass_kernel_spmd

F32 = mybir.dt.float32
I16 = mybir.dt.int16
AF = mybir.ActivationFunctionType
ALU = mybir.AluOpType

CFG = dict(N=50000, E=800000, G=256, DIN=64, DH=128, R=4, NLOC=12800,
           SUPW=4, GMAX=3072, HALF=25600)


def _fill_cfg(cfg):
    c = dict(cfg)
    c["NTOT"] = c["R"] * c["NLOC"]
    c["NW"] = c["NLOC"] // 128
    return c


def _edge_norms(cfg, src, dst):
    N = cfg["N"]
    deg_out = np.bincount(src, minlength=N).astype(np.float64)
    deg_in = np.bincount(dst, minlength=N).astype(np.float64)
    nsrc = np.clip(deg_out, 1.0, None) ** -0.5
    ndst = np.clip(deg_in, 1.0, None) ** -0.5
    return (nsrc[src] * ndst[dst]).astype(np.float32)


def _rank_edges(cfg, src, dst, w_all, rank):
    NLOC, HALF = cfg["NLOC"], cfg["HALF"]
    lo, hi = rank * NLOC, (rank + 1) * NLOC
    sel = (dst >= lo) & (dst < hi)
    s, d, w = src[sel], dst[sel], w_all[sel]
    order = np.argsort(d, kind="stable")
    s, d, w = s[order], d[order], w[order]
    win = (d - lo) // 128
    half = s // HALF
    out = {}
    for wi in np.unique(win):
        m = win == wi
        for h in (0, 1):
            mh = m & (half == h)
            if mh.any():
                out[(int(wi), h)] = (s[mh],
                                     (d[mh] - lo - wi * 128).astype(np.float32),
                                     w[mh])
    return out


def _build_schedule(cfg, per_core_edges):
    NW, SUPW, GMAX = cfg["NW"], cfg["SUPW"], cfg["GMAX"]
    nch_wh = {}
    for wi in range(NW):
        for h in (0, 1):
            mx = 0
            for pc in per_core_edges:
                if (wi, h) in pc:
                    mx = max(mx, (len(pc[(wi, h)][0]) + 127) // 128)
            if mx:
                nch_wh[(wi, h)] = mx

    chunk_win = []
    sw_instrs = []
    pos = 0
    for sw0 in range(0, NW, SUPW):
        wins = list(range(sw0, min(sw0 + SUPW, NW)))
        il = []
        for h in (0, 1):
            run_start = pos
            for wi in wins:
                k = nch_wh.get((wi, h), 0)
                chunk_win.extend([wi] * k)
                pos += k * 128
            st = run_start
            while st < pos:
                n = min(GMAX, pos - st)
                il.append((h, st, n))
                st += n
        sw_instrs.append(il)
    rows = pos
    chunk_win = np.asarray(chunk_win, np.int64)
    last_chunk = np.full(NW, -1, np.int64)
    for c, wi in enumerate(chunk_win):
        last_chunk[wi] = c

    per_core = []
    for pc in per_core_edges:
        gsrc = np.zeros(rows, np.int64)
        drel = np.full(rows, -1.0, np.float32)
        wv = np.zeros(rows, np.float32)
        p = 0
        for sw0 in range(0, NW, SUPW):
            wins = list(range(sw0, min(sw0 + SUPW, NW)))
            for h in (0, 1):
                for wi in wins:
                    k = nch_wh.get((wi, h), 0)
                    if not k:
                        continue
                    if (wi, h) in pc:
                        s, dr, w = pc[(wi, h)]
                        n = len(s)
                        gsrc[p:p + n] = s
                        drel[p:p + n] = dr
                        wv[p:p + n] = w
                    p += k * 128
        per_core.append(dict(gsrc=gsrc, drel=drel, w=wv))
    sched = dict(rows=rows, chunk_win=chunk_win, sw_instrs=sw_instrs,
                 last_chunk=last_chunk)
    return sched, per_core


def _wrap_idx16(gsrc, half_size):
    rows = len(gsrc)
    rel = (gsrc % half_size).astype(np.int16)
    blk = rel.reshape(rows // 16, 16).T
    return np.tile(blk, (8, 1)).copy()


def _mat128(vec):
    rows = len(vec)
    return np.ascontiguousarray(vec.reshape(rows // 128, 128).T)


def _build_nc(cfg, sched, b2_const, n_cores):
    N, E, G, DIN, DH, R, NLOC, NW, SUPW, GMAX, HALF, NTOT = (
        cfg[k] for k in ("N", "E", "G", "DIN", "DH", "R", "NLOC", "NW",
                         "SUPW", "GMAX", "HALF", "NTOT"))
    ROWS = sched["rows"]
    NCH = ROWS // 128
    chunk_win = sched["chunk_win"]
    last_chunk = sched["last_chunk"]
    sw_instrs = sched["sw_instrs"]

    nc = bacc.Bacc("TRN2", target_bir_lowering=False, debug=False,
                   enable_asserts=True, num_devices=n_cores)

    def dram(name, shape, dt=F32, kind="ExternalInput"):
        return nc.dram_tensor(name, shape, dt, kind=kind).ap()

    xT = dram("xT", [DIN, NTOT])
    gidx = dram("gidx", [128, ROWS // 16], I16)
    wmat = dram("wmat", [128, NCH])
    drmat = dram("drmat", [128, NCH])
    iota = dram("iota", [128, 128])
    iotaG = dram("iotaG", [128, G])
    gidrow = dram("gidrow", [128, NW])
    ones_row = dram("ones_row", [1, 128])
    W0 = dram("W0", [DIN, DH])
    W1 = dram("W1", [DH, DH])
    W2 = dram("W2", [DH, DH])
    biases = dram("biases", [3, DH])
    mW0su = dram("mW0su", [DH, 128])
    mW0sv = dram("mW0sv", [DH, 128])
    mW0gf = dram("mW0gf", [4, 128])
    mW1 = dram("mW1", [128, 64])
    mW2 = dram("mW2", [64, 1])
    b0c = dram("b0c", [128, 1])
    b1c = dram("b1c", [64, 1])
    gfT = dram("gfT", [4, G])
    icnt_su = dram("icnt_su", [128, G])
    icnt_sv = dram("icnt_sv", [128, G])
    y = dram("y", [G, 1], kind="ExternalOutput")

    with tile.TileContext(nc) as tc:
        with tc.tile_pool(name="const", bufs=1) as cpool, \
             tc.tile_pool(name="hT", bufs=1) as hpool, \
             tc.tile_pool(name="gath", bufs=3) as gpool, \
             tc.tile_pool(name="oneh", bufs=3) as opool, \
             tc.tile_pool(name="dense", bufs=3) as dpool, \
             tc.tile_pool(name="psc", bufs=SUPW + 1, space="PSUM") as psc, \
             tc.tile_pool(name="psd", bufs=3, space="PSUM") as psd, \
             tc.tile_pool(name="dram", bufs=1, space="DRAM") as drp:

            nc.gpsimd.load_library(mlp_lib)

            t_gidx = cpool.tile([128, ROWS // 16], I16)
            nc.sync.dma_start(out=t_gidx[:], in_=gidx[:])
            t_w = cpool.tile([128, NCH], F32)
            nc.sync.dma_start(out=t_w[:], in_=wmat[:])
            t_dr = cpool.tile([128, NCH], F32)
            nc.sync.dma_start(out=t_dr[:], in_=drmat[:])
            t_iota = cpool.tile([128, 128], F32)
            nc.sync.dma_start(out=t_iota[:], in_=iota[:])
            t_iotaG = cpool.tile([128, G], F32)
            nc.sync.dma_start(out=t_iotaG[:], in_=iotaG[:])
            t_gidrow = cpool.tile([128, NW], F32)
            nc.sync.dma_start(out=t_gidrow[:], in_=gidrow[:])
            t_ones = cpool.tile([1, 128], F32)
            nc.sync.dma_start(out=t_ones[:], in_=ones_row[:])
            t_ident = cpool.tile([128, 128], F32)
            make_identity(nc, t_ident[:])
            t_W = []
            for nm, ap_, k in (("w0", W0, DIN), ("w1", W1, DH), ("w2", W2, DH)):
                tw = cpool.tile([k, DH], F32, name=f"t_{nm}")
                nc.sync.dma_start(out=tw[:], in_=ap_[:])
                t_W.append(tw)
            t_bias = []
            for l in range(3):
                tb_l = cpool.tile([1, DH], F32, name=f"t_bias{l}")
                nc.sync.dma_start(out=tb_l[:], in_=biases[l:l + 1, :])
                t_bias.append(tb_l)

            t_hT = hpool.tile([128, NLOC], F32)

            btbl = [drp.tile([NTOT, DH], F32, name=f"btbl{i}") for i in range(2)]
            cin = drp.tile([128, NLOC], F32, name="cin")
            hTall = [drp.tile([R, 128, NLOC], F32, name=f"hTall{i}")
                     for i in range(2)]
            pool_cin = drp.tile([128, G], F32, name="pool_cin")
            pool_out = drp.tile([128, G], F32, name="pool_out")
            pair_cin = drp.tile([128, G], F32, name="pair_cin")
            pair_out = drp.tile([2, 128, G], F32, name="pair_out")

            group_a = [list(range(R)), list(range(R, 2 * R))]
            group_pairs = [[r, r + R] for r in range(R)]

            def dense(l):
                W = t_W[l]
                K = DIN if l == 0 else DH
                tbl = btbl[l % 2]
                for rb in range(R):
                    for c5 in range(NLOC // 512):
                        th = dpool.tile([K, 512], F32, name="th", tag="th")
                        if l == 0:
                            nc.sync.dma_start(
                                out=th[:],
                                in_=xT[:, rb * NLOC + c5 * 512:
                                       rb * NLOC + (c5 + 1) * 512])
                        else:
                            nc.sync.dma_start(
                                out=th[:],
                                in_=hTall[(l - 1) % 2][rb, :,
                                                       c5 * 512:(c5 + 1) * 512])
                        tb = dpool.tile([128, 4, 128], F32, name="tb", tag="tb")
                        for j in range(4):
                            pd = psd.tile([128, 128], F32, name="pd", tag="pd")
                            nc.tensor.matmul(out=pd[:],
                                             lhsT=th[:, j * 128:(j + 1) * 128],
                                             rhs=W[:], start=True, stop=True)
                            nc.scalar.activation(out=tb[:, j, :], in_=pd[:],
                                                 func=AF.Copy)
                        base = rb * NLOC + c5 * 512
                        nc.sync.dma_start(
                            out=tbl.tensor.ap()[base:base + 512, :]
                                .rearrange("(c p) d -> p c d", p=128),
                            in_=tb[:])

            def scatter(l):
                tbl = btbl[l % 2]
                for swi, sw0 in enumerate(range(0, NW, SUPW)):
                    wins = list(range(sw0, min(sw0 + SUPW, NW)))
                    pw = {}
                    for wi in wins:
                        pw[wi] = psc.tile([128, 128], F32,
                                          name=f"pw{wi % SUPW}", tag="pw")
                        nc.tensor.matmul(out=pw[wi][:], lhsT=t_bias[l][:],
                                         rhs=t_ones[:], start=True,
                                         stop=bool(last_chunk[wi] < 0))
                    for (half, st, n) in sw_instrs[swi]:
                        k = n // 128
                        tg = gpool.tile([128, GMAX // 128, 128], F32,
                                        name="tg", tag="tg")
                        nc.gpsimd.dma_gather(
                            out_ap=tg[:, :k, :],
                            in_ap=tbl.tensor.ap()[half * HALF:(half + 1) * HALF, :],
                            idxs_ap=t_gidx[:, st // 16:(st + n) // 16],
                            num_idxs=n, num_idxs_reg=n, elem_size=DH,
                            single_packet=False)
                        nc.vector.tensor_tensor(
                            out=tg[:, :k, :], in0=tg[:, :k, :],
                            in1=t_w[:, st // 128:st // 128 + k, None]
                                .to_broadcast([128, k, 128]),
                            op=ALU.mult)
                        toh = opool.tile([128, GMAX // 128, 128], F32,
                                         name="toh", tag="toh")
                        nc.vector.tensor_tensor(
                            out=toh[:, :k, :],
                            in0=t_dr[:, st // 128:st // 128 + k, None]
                                .to_broadcast([128, k, 128]),
                            in1=t_iota[:, None, :].to_broadcast([128, k, 128]),
                            op=ALU.is_equal)
                        for j in range(k):
                            ch = st // 128 + j
                            wi = int(chunk_win[ch])
                            nc.tensor.matmul(out=pw[wi][:], lhsT=tg[:, j, :],
                                             rhs=toh[:, j, :], start=False,
                                             stop=bool(ch == last_chunk[wi]))
                    for wi in wins:
                        nc.scalar.activation(
                            out=t_hT[:, wi * 128:(wi + 1) * 128], in_=pw[wi][:],
                            func=AF.Relu)

            for l in range(3):
                dense(l)
                scatter(l)
                if l < 2:
                    nc.sync.dma_start(out=cin[:], in_=t_hT[:])
                    nc.gpsimd.collective_compute(
                        "AllGather", ALU.bypass, replica_groups=group_a,
                        ins=[cin[:]], outs=[hTall[l % 2][:]])

            ppool = psd.tile([128, G], F32, name="ppool", tag="pd")
            for wi in range(NW):
                ptr = psd.tile([128, 128], F32, name="ptr", tag="pd")
                nc.tensor.transpose(out=ptr[:],
                                    in_=t_hT[:, wi * 128:(wi + 1) * 128],
                                    identity=t_ident[:])
                t_hrow = dpool.tile([128, 128], F32, name="t_hrow", tag="th")
                nc.scalar.activation(out=t_hrow[:], in_=ptr[:], func=AF.Copy)
                t_memb = dpool.tile([128, G], F32, name="t_memb", tag="tb")
                nc.vector.tensor_tensor(
                    out=t_memb[:],
                    in0=t_gidrow[:, wi:wi + 1].to_broadcast([128, G]),
                    in1=t_iotaG[:], op=ALU.is_equal)
                nc.tensor.matmul(out=ppool[:], lhsT=t_hrow[:], rhs=t_memb[:],
                                 start=wi == 0, stop=wi == NW - 1)
            t_pool = cpool.tile([128, G], F32)
            nc.scalar.activation(out=t_pool[:], in_=ppool[:], func=AF.Copy)
            nc.sync.dma_start(out=pool_cin[:], in_=t_pool[:])
            nc.gpsimd.collective_compute(
                "AllReduce", ALU.add, replica_groups=group_a,
                ins=[pool_cin[:]], outs=[pool_out[:]])
            t_pool2 = cpool.tile([128, G], F32)
            nc.sync.dma_start(out=t_pool2[:], in_=pool_out[:])
            nc.sync.dma_start(out=pair_cin[:], in_=t_pool2[:])
            nc.gpsimd.collective_compute(
                "AllGather", ALU.bypass, replica_groups=group_pairs,
                ins=[pair_cin[:]], outs=[pair_out[:]])

            t_su = cpool.tile([128, G], F32)
            t_sv = cpool.tile([128, G], F32)
            t_icsu = cpool.tile([128, G], F32)
            nc.sync.dma_start(out=t_icsu[:], in_=icnt_su[:])
            t_icsv = cpool.tile([128, G], F32)
            nc.sync.dma_start(out=t_icsv[:], in_=icnt_sv[:])
            t_su_raw = cpool.tile([128, G], F32)
            nc.sync.dma_start(out=t_su_raw[:], in_=pair_out[0])
            t_sv_raw = cpool.tile([128, G], F32)
            nc.sync.dma_start(out=t_sv_raw[:], in_=pair_out[1])
            nc.vector.tensor_tensor(out=t_su[:], in0=t_su_raw[:], in1=t_icsu[:],
                                    op=ALU.mult)
            nc.vector.tensor_tensor(out=t_sv[:], in0=t_sv_raw[:], in1=t_icsv[:],
                                    op=ALU.mult)
            t_gf = cpool.tile([4, G], F32)
            nc.sync.dma_start(out=t_gf[:], in_=gfT[:])
            t_mW0su = cpool.tile([DH, 128], F32)
            nc.sync.dma_start(out=t_mW0su[:], in_=mW0su[:])
            t_mW0sv = cpool.tile([DH, 128], F32)
            nc.sync.dma_start(out=t_mW0sv[:], in_=mW0sv[:])
            t_mW0gf = cpool.tile([4, 128], F32)
            nc.sync.dma_start(out=t_mW0gf[:], in_=mW0gf[:])
            t_mW1 = cpool.tile([128, 64], F32)
            nc.sync.dma_start(out=t_mW1[:], in_=mW1[:])
            t_mW2 = cpool.tile([64, 1], F32)
            nc.sync.dma_start(out=t_mW2[:], in_=mW2[:])
            t_b0c = cpool.tile([128, 1], F32)
            nc.sync.dma_start(out=t_b0c[:], in_=b0c[:])
            t_b1c = cpool.tile([64, 1], F32)
            nc.sync.dma_start(out=t_b1c[:], in_=b1c[:])

            ph1 = psd.tile([128, G], F32, name="ph1", tag="pd")
            nc.tensor.matmul(out=ph1[:], lhsT=t_mW0su[:], rhs=t_su[:],
                             start=True, stop=False)
            nc.tensor.matmul(out=ph1[:], lhsT=t_mW0sv[:], rhs=t_sv[:],
                             start=False, stop=False)
            nc.tensor.matmul(out=ph1[:], lhsT=t_mW0gf[:], rhs=t_gf[:],
                             start=False, stop=True)
            t_h1 = cpool.tile([128, G], F32)
            nc.scalar.activation(out=t_h1[:], in_=ph1[:], func=AF.Relu,
                                 bias=t_b0c[:, :1])
            ph2 = psd.tile([64, G], F32, name="ph2", tag="pd")
            nc.tensor.matmul(out=ph2[:], lhsT=t_mW1[:], rhs=t_h1[:],
                             start=True, stop=True)
            t_h2 = cpool.tile([64, G], F32)
            nc.scalar.activation(out=t_h2[:], in_=ph2[:], func=AF.Relu,
                                 bias=t_b1c[:, :1])
            po = psd.tile([1, G], F32, name="po", tag="pd")
            nc.tensor.matmul(out=po[:], lhsT=t_mW2[:], rhs=t_h2[:],
                             start=True, stop=True)
            t_o = cpool.tile([1, G], F32)
            nc.scalar.activation(out=t_o[:], in_=po[:], func=AF.Copy,
                                 bias=float(b2_const))
            nc.sync.dma_start(out=y[:], in_=t_o[:, :, None])

    nc.compile()
    return nc


def _host_prep(cfg, inputs):
    cfg = _fill_cfg(cfg)
    N, G, DIN, DH, R, NLOC, NW, NTOT, HALF = (
        cfg[k] for k in ("N", "G", "DIN", "DH", "R", "NLOC", "NW", "NTOT",
                         "HALF"))
    enc = []
    for pre in ("solute", "solvent"):
        src = np.asarray(inputs[f"{pre}_src"]).astype(np.int64)
        dst = np.asarray(inputs[f"{pre}_dst"]).astype(np.int64)
        gid = np.asarray(inputs[f"{pre}_gid"]).astype(np.int64)
        x = np.asarray(inputs[f"{pre}_x"], np.float32)
        w_all = _edge_norms(cfg, src, dst)
        enc.append(dict(src=src, dst=dst, gid=gid, x=x, w=w_all))

    per_core_edges = []
    for e in enc:
        for r in range(R):
            per_core_edges.append(_rank_edges(cfg, e["src"], e["dst"], e["w"], r))
    sched, pc_arrays = _build_schedule(cfg, per_core_edges)

    iota = np.broadcast_to(np.arange(128, dtype=np.float32), (128, 128)).copy()
    iotaG = np.broadcast_to(np.arange(G, dtype=np.float32), (128, G)).copy()
    ones_row = np.ones((1, 128), np.float32)
    gfT = np.ascontiguousarray(np.asarray(inputs["global_feats"], np.float32).T)
    mW0 = np.asarray(inputs["mlp_W0"], np.float32)
    icnts = []
    for e in enc:
        cnt = np.maximum(np.bincount(e["gid"], minlength=G), 1.0).astype(np.float32)
        icnts.append(np.broadcast_to(1.0 / cnt, (128, G)).copy())
    b2_const = float(np.asarray(inputs["mlp_b2"]).reshape(-1)[0])

    xTs, gidrows = [], []
    for e in enc:
        xp = np.zeros((NTOT, DIN), np.float32)
        xp[:N] = e["x"]
        xTs.append(np.ascontiguousarray(xp.T))
        gr = np.full(NTOT, -1.0, np.float32)
        gr[:N] = e["gid"].astype(np.float32)
        gidrows.append(gr)

    in_maps = []
    for gi in range(2):
        pre = "su" if gi == 0 else "sv"
        for r in range(R):
            c = gi * R + r
            arr = pc_arrays[c]
            gr_loc = gidrows[gi][r * NLOC:(r + 1) * NLOC]
            im = dict(
                xT=xTs[gi],
                gidx=_wrap_idx16(arr["gsrc"], HALF),
                wmat=_mat128(arr["w"]),
                drmat=_mat128(arr["drel"]),
                iota=iota, iotaG=iotaG,
                gidrow=_mat128(gr_loc),
                ones_row=ones_row,
                W0=np.asarray(inputs[f"{pre}_W0"], np.float32),
                W1=np.asarray(inputs[f"{pre}_W1"], np.float32),
                W2=np.asarray(inputs[f"{pre}_W2"], np.float32),
                biases=np.asarray(inputs[f"{pre}_b"], np.float32),
                mW0su=np.ascontiguousarray(mW0[0:DH, :]),
                mW0sv=np.ascontiguousarray(mW0[DH:2 * DH, :]),
                mW0gf=np.ascontiguousarray(mW0[2 * DH:2 * DH + 4, :]),
                mW1=np.asarray(inputs["mlp_W1"], np.float32),
                mW2=np.asarray(inputs["mlp_W2"], np.float32),
                b0c=np.asarray(inputs["mlp_b0"], np.float32).reshape(128, 1),
                b1c=np.asarray(inputs["mlp_b1"], np.float32).reshape(64, 1),
                gfT=gfT, icnt_su=icnts[0], icnt_sv=icnts[1],
            )
            in_maps.append(im)
    return cfg, sched, b2_const, in_maps


_CACHE = {}


def kernel(**inputs) -> np.ndarray:
    cfg, sched, b2c, in_maps = _host_prep(CFG, inputs)
    key = (sched["rows"], b2c, sched["chunk_win"].tobytes(),
           tuple(i for sw in sched["sw_instrs"] for i in sw))
    nc = _CACHE.get(key)
    if nc is None:
        nc = _build_nc(cfg, sched, b2c, 8)
        _CACHE[key] = nc
    res = run_bass_kernel_spmd(nc, in_maps, core_ids=list(range(8)))
    return np.asarray(res.results[0]["y"], np.float32)


# revision 3
# speedup vs baseline: 1.7313x; 1.7313x over previous
"""DualGNNModel Trainium2 kernel (8 NeuronCores, Bass/Tile).

Self-contained: accepts FULL inputs (as reference.setup_inputs()), returns the
FULL [256, 1] float32 output.

Sharding: cores 0-3 run the solute GCN encoder, cores 4-7 the solvent encoder
(graph-level model parallelism over the two independent encoders). Within each
4-core group, edges are partitioned by destination node into 4 contiguous
12800-node ranges; GCN/MLP weights are replicated. Per layer each core:
  dense   b = h @ W                  (replicated over the group)
  gather  g = b[src] rows            (dma_gather, two int16 half-table passes)
  scale   g *= nsrc[src]*ndst[dst]   (symmetric GCN norm folded per edge)
  scatter psum[feat, dstwin] += g_chunk^T @ onehot(dst)    (PE matmuls)
  bias    psum += outer(bias, ones);  hT_own = relu(psum)
h shards are AllGather'd between layers; after layer 3 each core pools its own
shard via a membership-one-hot matmul, partial pools are AllReduce'd within the
group, the two encoders' pooled embeddings are exchanged pairwise, and the
(tiny) MLP head runs replicated on every core.
"""
import numpy as np
import concourse.bass as bass
import concourse.bacc as bacc
import concourse.mybir as mybir
import concourse.tile as tile
from concourse.library_config import mlp as mlp_lib
from concourse.masks import make_identity
from concourse.bass_utils import run_bass_kernel_spmd

F32 = mybir.dt.float32
I16 = mybir.dt.int16
AF = mybir.ActivationFunctionType
ALU = mybir.AluOpType

CFG = dict(N=50000, E=800000, G=256, DIN=64, DH=128, R=4, NLOC=12800,
           SUPW=4, GMAX=3072, HALF=25600)


def _fill_cfg(cfg):
    c = dict(cfg)
    c["NTOT"] = c["R"] * c["NLOC"]
    c["NW"] = c["NLOC"] // 128
    return c


def _edge_norms(cfg, src, dst):
    N = cfg["N"]
    deg_out = np.bincount(src, minlength=N).astype(np.float64)
    deg_in = np.bincount(dst, minlength=N).astype(np.float64)
    nsrc = np.clip(deg_out, 1.0, None) ** -0.5
    ndst = np.clip(deg_in, 1.0, None) ** -0.5
    return (nsrc[src] * ndst[dst]).astype(np.float32)


def _rank_edges(cfg, src, dst, w_all, rank):
    NLOC, HALF = cfg["NLOC"], cfg["HALF"]
    lo, hi = rank * NLOC, (rank + 1) * NLOC
    sel = (dst >= lo) & (dst < hi)
    s, d, w = src[sel], dst[sel], w_all[sel]
    order = np.argsort(d, kind="stable")
    s, d, w = s[order], d[order], w[order]
    win = (d - lo) // 128
    half = s // HALF
    out = {}
    for wi in np.unique(win):
        m = win == wi
        for h in (0, 1):
            mh = m & (half == h)
            if mh.any():
                out[(int(wi), h)] = (s[mh],
                                     (d[mh] - lo - wi * 128).astype(np.float32),
                                     w[mh])
    return out


def _build_schedule(cfg, per_core_edges):
    NW, SUPW, GMAX = cfg["NW"], cfg["SUPW"], cfg["GMAX"]
    nch_wh = {}
    for wi in range(NW):
        for h in (0, 1):
            mx = 0
            for pc in per_core_edges:
                if (wi, h) in pc:
                    mx = max(mx, (len(pc[(wi, h)][0]) + 127) // 128)
            if mx:
                nch_wh[(wi, h)] = mx

    chunk_win = []
    sw_instrs = []
    pos = 0
    for sw0 in range(0, NW, SUPW):
        wins = list(range(sw0, min(sw0 + SUPW, NW)))
        il = []
        for h in (0, 1):
            run_start = pos
            for wi in wins:
                k = nch_wh.get((wi, h), 0)
                chunk_win.extend([wi] * k)
                pos += k * 128
            st = run_start
            while st < pos:
                n = min(GMAX, pos - st)
                il.append((h, st, n))
                st += n
        sw_instrs.append(il)
    rows = pos
    chunk_win = np.asarray(chunk_win, np.int64)
    last_chunk = np.full(NW, -1, np.int64)
    for c, wi in enumerate(chunk_win):
        last_chunk[wi] = c

    per_core = []
    for pc in per_core_edges:
        gsrc = np.zeros(rows, np.int64)
        drel = np.full(rows, -1.0, np.float32)
        wv = np.zeros(rows, np.float32)
        p = 0
        for sw0 in range(0, NW, SUPW):
            wins = list(range(sw0, min(sw0 + SUPW, NW)))
            for h in (0, 1):
                for wi in wins:
                    k = nch_wh.get((wi, h), 0)
                    if not k:
                        continue
                    if (wi, h) in pc:
                        s, dr, w = pc[(wi, h)]
                        n = len(s)
                        gsrc[p:p + n] = s
                        drel[p:p + n] = dr
                        wv[p:p + n] = w
                    p += k * 128
        per_core.append(dict(gsrc=gsrc, drel=drel, w=wv))
    sched = dict(rows=rows, chunk_win=chunk_win, sw_instrs=sw_instrs,
                 last_chunk=last_chunk)
    return sched, per_core


def _wrap_idx16(gsrc, half_size):
    rows = len(gsrc)
    rel = (gsrc % half_size).astype(np.int16)
    blk = rel.reshape(rows // 16, 16).T
    return np.tile(blk, (8, 1)).copy()


def _mat128(vec):
    rows = len(vec)
    return np.ascontiguousarray(vec.reshape(rows // 128, 128).T)


def _build_nc(cfg, sched, b2_const, n_cores, nrep=1):
    N, E, G, DIN, DH, R, NLOC, NW, SUPW, GMAX, HALF, NTOT = (
        cfg[k] for k in ("N", "E", "G", "DIN", "DH", "R", "NLOC", "NW",
                         "SUPW", "GMAX", "HALF", "NTOT"))
    ROWS = sched["rows"]
    NCH = ROWS // 128
    chunk_win = sched["chunk_win"]
    last_chunk = sched["last_chunk"]
    sw_instrs = sched["sw_instrs"]

    nc = bacc.Bacc("TRN2", target_bir_lowering=False, debug=False,
                   enable_asserts=True, num_devices=n_cores)

    def dram(name, shape, dt=F32, kind="ExternalInput"):
        return nc.dram_tensor(name, shape, dt, kind=kind).ap()

    xT = dram("xT", [DIN, NTOT])
    gidx = dram("gidx", [128, ROWS // 16], I16)
    wmat = dram("wmat", [128, NCH])
    drmat = dram("drmat", [128, NCH])
    iota = dram("iota", [128, 128])
    iotaG = dram("iotaG", [128, G])
    gidrow = dram("gidrow", [128, NW])
    ones_row = dram("ones_row", [1, 128])
    W0 = dram("W0", [DIN, DH])
    W1 = dram("W1", [DH, DH])
    W2 = dram("W2", [DH, DH])
    biases = dram("biases", [3, DH])
    mW0su = dram("mW0su", [DH, 128])
    mW0sv = dram("mW0sv", [DH, 128])
    mW0gf = dram("mW0gf", [4, 128])
    mW1 = dram("mW1", [128, 64])
    mW2 = dram("mW2", [64, 1])
    b0c = dram("b0c", [128, 1])
    b1c = dram("b1c", [64, 1])
    gfT = dram("gfT", [4, G])
    icnt_su = dram("icnt_su", [128, G])
    icnt_sv = dram("icnt_sv", [128, G])
    y = dram("y", [G, 1], kind="ExternalOutput")

    with tile.TileContext(nc) as tc:
        with tc.tile_pool(name="const", bufs=1) as cpool, \
             tc.tile_pool(name="hT", bufs=1) as hpool, \
             tc.tile_pool(name="gath", bufs=3) as gpool, \
             tc.tile_pool(name="oneh", bufs=3) as opool, \
             tc.tile_pool(name="dense", bufs=3) as dpool, \
             tc.tile_pool(name="psc", bufs=SUPW + 1, space="PSUM") as psc, \
             tc.tile_pool(name="psd", bufs=3, space="PSUM") as psd, \
             tc.tile_pool(name="dram", bufs=1, space="DRAM") as drp:

            nc.gpsimd.load_library(mlp_lib)

            t_gidx = cpool.tile([128, ROWS // 16], I16)
            nc.sync.dma_start(out=t_gidx[:], in_=gidx[:])
            t_w = cpool.tile([128, NCH], F32)
            nc.sync.dma_start(out=t_w[:], in_=wmat[:])
            t_dr = cpool.tile([128, NCH], F32)
            nc.sync.dma_start(out=t_dr[:], in_=drmat[:])
            t_iota = cpool.tile([128, 128], F32)
            nc.sync.dma_start(out=t_iota[:], in_=iota[:])
            t_iotaG = cpool.tile([128, G], F32)
            nc.sync.dma_start(out=t_iotaG[:], in_=iotaG[:])
            t_gidrow = cpool.tile([128, NW], F32)
            nc.sync.dma_start(out=t_gidrow[:], in_=gidrow[:])
            t_ones = cpool.tile([1, 128], F32)
            nc.sync.dma_start(out=t_ones[:], in_=ones_row[:])
            t_ident = cpool.tile([128, 128], F32)
            make_identity(nc, t_ident[:])
            t_W = []
            for nm, ap_, k in (("w0", W0, DIN), ("w1", W1, DH), ("w2", W2, DH)):
                tw = cpool.tile([k, DH], F32, name=f"t_{nm}")
                nc.sync.dma_start(out=tw[:], in_=ap_[:])
                t_W.append(tw)
            t_bias = []
            def one_pass(rep):
                sfx = f"_{rep}"
                for l in range(3):
                    tb_l = cpool.tile([1, DH], F32, name=f"t_bias{l}")
                    nc.sync.dma_start(out=tb_l[:], in_=biases[l:l + 1, :])
                    t_bias.append(tb_l)

                t_hT = hpool.tile([128, NLOC], F32)

                btbl = [[drp.tile([HALF, DH], F32, name=f"btbl{i}h{h}") for h in range(2)]
                    for i in range(2)]
                NSEG = 5
                SEGW = NW // NSEG
                SEGN = SEGW * 128
                cinq = [drp.tile([128, SEGN], F32, name=f"cin{q}" + sfx)
                        for q in range(NSEG)]
                hTall = [[drp.tile([R, 128, SEGN], F32, name=f"hTall{i}q{q}" + sfx)
                          for q in range(NSEG)] for i in range(2)]
                pool_cin = drp.tile([128, G], F32, name="pool_cin")
                pool_out = drp.tile([128, G], F32, name="pool_out")
                pair_cin = drp.tile([128, G], F32, name="pair_cin")
                pair_out = drp.tile([2, 128, G], F32, name="pair_out")

                group_a = [list(range(R)), list(range(R, 2 * R))]
                group_pairs = [[r, r + R] for r in range(R)]

                def dense(l):
                    W = t_W[l]
                    K = DIN if l == 0 else DH
                    tbl = btbl[l % 2]
                    for rb in range(R):
                        for c5 in range(NLOC // 512):
                                th = dpool.tile([K, 512], F32, name="th", tag="th")
                                if l == 0:
                                    nc.sync.dma_start(
                                        out=th[:],
                                        in_=xT[:, rb * NLOC + c5 * 512:
                                                   rb * NLOC + (c5 + 1) * 512])
                                else:
                                    q = (c5 * 512) // SEGN
                                    off = (c5 * 512) % SEGN
                                    nc.sync.dma_start(
                                        out=th[:],
                                        in_=hTall[(l - 1) % 2][q][rb, :, off:off + 512])
                                tb = dpool.tile([128, 4, 128], F32, name="tb", tag="tb")
                                for j in range(4):
                                    pd = psd.tile([128, 128], F32, name="pd", tag="pd")
                                    nc.tensor.matmul(out=pd[:],
                                                         lhsT=th[:, j * 128:(j + 1) * 128],
                                                         rhs=W[:], start=True, stop=True)
                                    nc.scalar.activation(out=tb[:, j, :], in_=pd[:],
                                                                 func=AF.Copy)
                                gbase = rb * NLOC + c5 * 512
                                tb_h = tbl[gbase // HALF]
                                base = gbase % HALF
                                nc.sync.dma_start(
                                    out=tb_h.tensor.ap()[base:base + 512, :]
                                        .rearrange("(c p) d -> p c d", p=128),
                                    in_=tb[:])

                def scatter(l, do_ag):
                    tbl = btbl[l % 2]
                    for swi, sw0 in enumerate(range(0, NW, SUPW)):
                        wins = list(range(sw0, min(sw0 + SUPW, NW)))
                        pw = {}
                        for wi in wins:
                                pw[wi] = psc.tile([128, 128], F32,
                                                      name=f"pw{wi % SUPW}", tag="pw")
                                nc.tensor.matmul(out=pw[wi][:], lhsT=t_bias[l][:],
                                                     rhs=t_ones[:], start=True,
                                                     stop=bool(last_chunk[wi] < 0))
                        for (half, st, n) in sw_instrs[swi]:
                                k = n // 128
                                tg = gpool.tile([128, GMAX // 128, 128], F32,
                                                    name="tg", tag="tg")
                                nc.gpsimd.dma_gather(
                                    out_ap=tg[:, :k, :],
                                    in_ap=tbl[half].tensor.ap()[:],
                                    idxs_ap=t_gidx[:, st // 16:(st + n) // 16],
                                    num_idxs=n, num_idxs_reg=n, elem_size=DH,
                                    single_packet=False)
                                nc.vector.tensor_tensor(
                                    out=tg[:, :k, :], in0=tg[:, :k, :],
                                    in1=t_w[:, st // 128:st // 128 + k, None]
                                        .to_broadcast([128, k, 128]),
                                    op=ALU.mult)
                                toh = opool.tile([128, GMAX // 128, 128], F32,
                                                     name="toh", tag="toh")
                                nc.vector.tensor_tensor(
                                    out=toh[:, :k, :],
                                    in0=t_dr[:, st // 128:st // 128 + k, None]
                                        .to_broadcast([128, k, 128]),
                                    in1=t_iota[:, None, :].to_broadcast([128, k, 128]),
                                    op=ALU.is_equal)
                                for j in range(k):
                                    ch = st // 128 + j
                                    wi = int(chunk_win[ch])
                                    nc.tensor.matmul(out=pw[wi][:], lhsT=tg[:, j, :],
                                                         rhs=toh[:, j, :], start=False,
                                                         stop=bool(ch == last_chunk[wi]))
                        for wi in wins:
                                nc.scalar.activation(
                                    out=t_hT[:, wi * 128:(wi + 1) * 128], in_=pw[wi][:],
                                    func=AF.Relu)
                                if do_ag and (wi + 1) % SEGW == 0:
                                    q = (wi + 1) // SEGW - 1
                                    nc.sync.dma_start(
                                        out=cinq[q][:],
                                        in_=t_hT[:, q * SEGN:(q + 1) * SEGN])
                                    nc.gpsimd.collective_compute(
                                        "AllGather", ALU.bypass,
                                        replica_groups=group_a,
                                        ins=[cinq[q][:]],
                                        outs=[hTall[l % 2][q][:]])

                for l in range(3):
                    dense(l)
                    scatter(l, do_ag=l < 2)

                ppool = psd.tile([128, G], F32, name="ppool" + sfx, tag="pd")
                for wi in range(NW):
                    ptr = psd.tile([128, 128], F32, name="ptr" + sfx, tag="pd")
                    nc.tensor.transpose(out=ptr[:],
                                                in_=t_hT[:, wi * 128:(wi + 1) * 128],
                                                identity=t_ident[:])
                    t_hrow = dpool.tile([128, 128], F32, name="t_hrow" + sfx, tag="th")
                    nc.scalar.activation(out=t_hrow[:], in_=ptr[:], func=AF.Copy)
                    t_memb = dpool.tile([128, G], F32, name="t_memb" + sfx, tag="tb")
                    nc.vector.tensor_tensor(
                        out=t_memb[:],
                        in0=t_gidrow[:, wi:wi + 1].to_broadcast([128, G]),
                        in1=t_iotaG[:], op=ALU.is_equal)
                    nc.tensor.matmul(out=ppool[:], lhsT=t_hrow[:], rhs=t_memb[:],
                                         start=wi == 0, stop=wi == NW - 1)
                t_pool = cpool.tile([128, G], F32, name="t_pool" + sfx)
                nc.scalar.activation(out=t_pool[:], in_=ppool[:], func=AF.Copy)
                nc.sync.dma_start(out=pool_cin[:], in_=t_pool[:])
                nc.gpsimd.collective_compute(
                    "AllReduce", ALU.add, replica_groups=group_a,
                    ins=[pool_cin[:]], outs=[pool_out[:]])
                t_pool2 = cpool.tile([128, G], F32, name="t_pool2" + sfx)
                nc.sync.dma_start(out=t_pool2[:], in_=pool_out[:])
                nc.sync.dma_start(out=pair_cin[:], in_=t_pool2[:])
                nc.gpsimd.collective_compute(
                    "AllGather", ALU.bypass, replica_groups=group_pairs,
                    ins=[pair_cin[:]], outs=[pair_out[:]])

                t_su = cpool.tile([128, G], F32, name="t_su" + sfx)
                t_sv = cpool.tile([128, G], F32, name="t_sv" + sfx)
                t_icsu = cpool.tile([128, G], F32, name="t_icsu" + sfx)
                nc.sync.dma_start(out=t_icsu[:], in_=icnt_su[:])
                t_icsv = cpool.tile([128, G], F32, name="t_icsv" + sfx)
                nc.sync.dma_start(out=t_icsv[:], in_=icnt_sv[:])
                t_su_raw = cpool.tile([128, G], F32, name="t_su_raw" + sfx)
                nc.sync.dma_start(out=t_su_raw[:], in_=pair_out[0])
                t_sv_raw = cpool.tile([128, G], F32, name="t_sv_raw" + sfx)
                nc.sync.dma_start(out=t_sv_raw[:], in_=pair_out[1])
                nc.vector.tensor_tensor(out=t_su[:], in0=t_su_raw[:], in1=t_icsu[:],
                                                op=ALU.mult)
                nc.vector.tensor_tensor(out=t_sv[:], in0=t_sv_raw[:], in1=t_icsv[:],
                                                op=ALU.mult)
                t_gf = cpool.tile([4, G], F32, name="t_gf" + sfx)
                nc.sync.dma_start(out=t_gf[:], in_=gfT[:])
                t_mW0su = cpool.tile([DH, 128], F32, name="t_mW0su" + sfx)
                nc.sync.dma_start(out=t_mW0su[:], in_=mW0su[:])
                t_mW0sv = cpool.tile([DH, 128], F32, name="t_mW0sv" + sfx)
                nc.sync.dma_start(out=t_mW0sv[:], in_=mW0sv[:])
                t_mW0gf = cpool.tile([4, 128], F32, name="t_mW0gf" + sfx)
                nc.sync.dma_start(out=t_mW0gf[:], in_=mW0gf[:])
                t_mW1 = cpool.tile([128, 64], F32, name="t_mW1" + sfx)
                nc.sync.dma_start(out=t_mW1[:], in_=mW1[:])
                t_mW2 = cpool.tile([64, 1], F32, name="t_mW2" + sfx)
                nc.sync.dma_start(out=t_mW2[:], in_=mW2[:])
                t_b0c = cpool.tile([128, 1], F32, name="t_b0c" + sfx)
                nc.sync.dma_start(out=t_b0c[:], in_=b0c[:])
                t_b1c = cpool.tile([64, 1], F32, name="t_b1c" + sfx)
                nc.sync.dma_start(out=t_b1c[:], in_=b1c[:])

                ph1 = psd.tile([128, G], F32, name="ph1" + sfx, tag="pd")
                nc.tensor.matmul(out=ph1[:], lhsT=t_mW0su[:], rhs=t_su[:],
                                     start=True, stop=False)
                nc.tensor.matmul(out=ph1[:], lhsT=t_mW0sv[:], rhs=t_sv[:],
                                     start=False, stop=False)
                nc.tensor.matmul(out=ph1[:], lhsT=t_mW0gf[:], rhs=t_gf[:],
                                     start=False, stop=True)
                t_h1 = cpool.tile([128, G], F32, name="t_h1" + sfx)
                nc.scalar.activation(out=t_h1[:], in_=ph1[:], func=AF.Relu,
                                         bias=t_b0c[:, :1])
                ph2 = psd.tile([64, G], F32, name="ph2" + sfx, tag="pd")
                nc.tensor.matmul(out=ph2[:], lhsT=t_mW1[:], rhs=t_h1[:],
                                     start=True, stop=True)
                t_h2 = cpool.tile([64, G], F32, name="t_h2" + sfx)
                nc.scalar.activation(out=t_h2[:], in_=ph2[:], func=AF.Relu,
                                         bias=t_b1c[:, :1])
                po = psd.tile([1, G], F32, name="po" + sfx, tag="pd")
                nc.tensor.matmul(out=po[:], lhsT=t_mW2[:], rhs=t_h2[:],
                                     start=True, stop=True)
                t_o = cpool.tile([1, G], F32, name="t_o" + sfx)
                nc.scalar.activation(out=t_o[:], in_=po[:], func=AF.Copy,
                                         bias=float(b2_const))
                nc.sync.dma_start(out=y[:], in_=t_o[:, :, None])

            for rep in range(nrep):
                one_pass(rep)

    nc.compile()
    return nc


def _host_prep(cfg, inputs):
    cfg = _fill_cfg(cfg)
    N, G, DIN, DH, R, NLOC, NW, NTOT, HALF = (
        cfg[k] for k in ("N", "G", "DIN", "DH", "R", "NLOC", "NW", "NTOT",
                         "HALF"))
    enc = []
    for pre in ("solute", "solvent"):
        src = np.asarray(inputs[f"{pre}_src"]).astype(np.int64)
        dst = np.asarray(inputs[f"{pre}_dst"]).astype(np.int64)
        gid = np.asarray(inputs[f"{pre}_gid"]).astype(np.int64)
        x = np.asarray(inputs[f"{pre}_x"], np.float32)
        w_all = _edge_norms(cfg, src, dst)
        enc.append(dict(src=src, dst=dst, gid=gid, x=x, w=w_all))

    per_core_edges = []
    for e in enc:
        for r in range(R):
            per_core_edges.append(_rank_edges(cfg, e["src"], e["dst"], e["w"], r))
    sched, pc_arrays = _build_schedule(cfg, per_core_edges)

    iota = np.broadcast_to(np.arange(128, dtype=np.float32), (128, 128)).copy()
    iotaG = np.broadcast_to(np.arange(G, dtype=np.float32), (128, G)).copy()
    ones_row = np.ones((1, 128), np.float32)
    gfT = np.ascontiguousarray(np.asarray(inputs["global_feats"], np.float32).T)
    mW0 = np.asarray(inputs["mlp_W0"], np.float32)
    icnts = []
    for e in enc:
        cnt = np.maximum(np.bincount(e["gid"], minlength=G), 1.0).astype(np.float32)
        icnts.append(np.broadcast_to(1.0 / cnt, (128, G)).copy())
    b2_const = float(np.asarray(inputs["mlp_b2"]).reshape(-1)[0])

    xTs, gidrows = [], []
    for e in enc:
        xp = np.zeros((NTOT, DIN), np.float32)
        xp[:N] = e["x"]
        xTs.append(np.ascontiguousarray(xp.T))
        gr = np.full(NTOT, -1.0, np.float32)
        gr[:N] = e["gid"].astype(np.float32)
        gidrows.append(gr)

    in_maps = []
    for gi in range(2):
        pre = "su" if gi == 0 else "sv"
        for r in range(R):
            c = gi * R + r
            arr = pc_arrays[c]
            gr_loc = gidrows[gi][r * NLOC:(r + 1) * NLOC]
            im = dict(
                xT=xTs[gi],
                gidx=_wrap_idx16(arr["gsrc"], HALF),
                wmat=_mat128(arr["w"]),
                drmat=_mat128(arr["drel"]),
                iota=iota, iotaG=iotaG,
                gidrow=_mat128(gr_loc),
                ones_row=ones_row,
                W0=np.asarray(inputs[f"{pre}_W0"], np.float32),
                W1=np.asarray(inputs[f"{pre}_W1"], np.float32),
                W2=np.asarray(inputs[f"{pre}_W2"], np.float32),
                biases=np.asarray(inputs[f"{pre}_b"], np.float32),
                mW0su=np.ascontiguousarray(mW0[0:DH, :]),
                mW0sv=np.ascontiguousarray(mW0[DH:2 * DH, :]),
                mW0gf=np.ascontiguousarray(mW0[2 * DH:2 * DH + 4, :]),
                mW1=np.asarray(inputs["mlp_W1"], np.float32),
                mW2=np.asarray(inputs["mlp_W2"], np.float32),
                b0c=np.asarray(inputs["mlp_b0"], np.float32).reshape(128, 1),
                b1c=np.asarray(inputs["mlp_b1"], np.float32).reshape(64, 1),
                gfT=gfT, icnt_su=icnts[0], icnt_sv=icnts[1],
            )
            in_maps.append(im)
    return cfg, sched, b2_const, in_maps


_CACHE = {}


def kernel(**inputs) -> np.ndarray:
    cfg, sched, b2c, in_maps = _host_prep(CFG, inputs)
    key = (sched["rows"], b2c, sched["chunk_win"].tobytes(),
           tuple(i for sw in sched["sw_instrs"] for i in sw))
    nc = _CACHE.get(key)
    if nc is None:
        nc = _build_nc(cfg, sched, b2c, 8)
        _CACHE[key] = nc
    res = run_bass_kernel_spmd(nc, in_maps, core_ids=list(range(8)))
    return np.asarray(res.results[0]["y"], np.float32)
